# revision 36
# baseline (speedup 1.0000x reference)
"""GCN (2x GCNConv + MLP head + softmax) on 8 TRN2 NeuronCores.

Strategy (graph/data parallel, per sharding hint):
  - Nodes sharded across 8 cores (2500 rows each, padded to 2560); weights
    replicated. Edges partitioned by dst; aggregation runs per dst window
    (128 slots) as dma_gather row gathers (4 SWDGE queues; the gather
    stream is the roofline resource at ~43 GB/s/queue) + one-hot segment
    matmuls on the TensorEngine accumulating in PSUM; fp8 chunk pairs run
    as DoubleRow matmuls (virtual K=256). One-hot matrices are built
    on-chip (DVE iota==slot); self-loops never gather (identity matmul of
    resident rows); trailing pad slots carry idx=-1 so the gather ucode
    skips their descriptors (num_idxs_reg = per-call live count, equalized
    across cores so the SPMD program stays uniform).
  - Layer 1 needs NO AllGather: aggregation commutes with the dense matmul
    (Ahat @ (x W1) == (Ahat @ x) W1), so the host stages the full
    dinv-scaled x as an fp8 table and gathers start immediately. Per
    window: aggregate -> dinv-scale on the PSUM->SBUF copy -> transpose ->
    dense @W1 emitting z1T blocks directly (bias+relu on the Scalar
    engine), software-pipelined one window behind the aggregation.
  - Layer 2 AllGathers the dinv-scaled h2 table in two parts (A=2048
    rows/core fired after z1 window 15, B=512 after window 19) so ~80% of
    its gathers never wait on the last z1 windows. Pass 1 aggregates
    region-A sources for all windows (spilling partials to SBUF), pass 2
    re-adds the spill via identity matmul, folds the bias in as a rank-1
    (sqrt(deg) x bias) matmul, and fuses the final dinv scaling into the
    Scalar-engine relu.
  - Head: two dense layers + row softmax per 128-row window of layer 2.

Host-side preprocessing is limited to graph-structure work (edge sort,
degree counts, window slots, gather-index layout, dinv*x staging).
"""

import os
from contextlib import ExitStack

import numpy as np

import concourse.bacc as bacc
import concourse.mybir as mybir
import concourse.tile as tile
from concourse.bass_utils import run_bass_kernel_spmd
from concourse.masks import make_identity

# problem shapes (hardcoded per contract)
N = 20000
E = 320000
D = 512
D_OUT = 128
NCORES = 8
RPC = 2500          # real rows per core
RPAD = 2560         # padded rows per core (20 tiles of 128)
NPAD = RPAD * NCORES
MT = RPAD // 128    # m-tiles / dst windows per core (20)
SPLIT2 = int(os.environ.get("GNN_SPLIT2", "1280"))  # L2 AG part-A rows/core
RA2 = SPLIT2 * NCORES
MA2 = SPLIT2 // 128
G = 6               # max chunks (of 128 edges) per dma_gather call
NQ = 4              # SWDGE queues for gather rotation

# config: "f32" (exact), "f32r" (fast fp32 matmul), "bf16" (half-traffic),
# "fp8" (bf16 compute, fp8e4 gathered tables: quarter gather/AG traffic)
MODE = os.environ.get("GNN_MODE", "fp8")

_f32 = mybir.dt.float32
_f32r = mybir.dt.float32r
_bf16 = mybir.dt.bfloat16
_f8 = mybir.dt.float8e4
_i16 = mybir.dt.int16
_i32 = mybir.dt.int32


def _table_l1(node):
    """Row of node in the host-staged xs table (padded core-major layout)."""
    return (node // RPC) * RPAD + (node % RPC)


def _table_l2(node):
    """Row of node in the L2 AllGather-ed table (part A then part B)."""
    c, r = node // RPC, node % RPC
    h = r >= SPLIT2
    return np.where(h, RA2 + c * (RPAD - SPLIT2) + (r - SPLIT2), c * SPLIT2 + r)


def _plan(tab_ids, dsts, core_bounds, nh, HB):
    """Window/chunk/call plan for one layer's gathers.

    tab_ids: per-edge table row (edges sorted by dst); nh: number of table
    regions (1 for L1, 2 for L2); HB: region-A size (region boundary).
    Returns dict with group_sizes[w][h], chunk_base[w][h], TC,
    vmap[(w,h,ci)], and per-core idx/wsl arrays.
    """
    counts = np.zeros((NCORES, MT, nh), dtype=np.int64)
    for c in range(NCORES):
        lo, hi = core_bounds[c], core_bounds[c + 1]
        d = dsts[lo:hi] - c * RPC
        hvec = (tab_ids[lo:hi] >= HB).astype(np.int64) if nh == 2 else None
        wb = np.searchsorted(d, np.arange(MT + 1) * 128)
        for w in range(MT):
            a, b = wb[w], wb[w + 1]
            if nh == 2:
                n1 = int(hvec[a:b].sum())
                counts[c, w, 0] = (b - a) - n1
                counts[c, w, 1] = n1
            else:
                counts[c, w, 0] = b - a

    cpw = np.maximum(1, -(-counts.max(axis=0) // 128))  # [MT, nh] chunks
    TC = int(cpw.sum())
    cb = np.concatenate([[0], np.cumsum(cpw.reshape(-1))]).astype(int)
    chunk_base = cb[:-1].reshape(MT, nh)

    group_sizes = []
    vmap = {}
    for w in range(MT):
        gw = []
        for h in range(nh):
            n = int(cpw[w, h])
            k = -(-n // G)
            base, rem = divmod(n, k)
            gs = [base + (i < rem) for i in range(k)]
            gw.append(gs)
            # gather every slot (pads hit row 0, masked by wsl=-1): ~4%
            # extra rows, but msg buffers then never hold stale bytes, so
            # the startup memset pass is unnecessary
            for ci, gsz in enumerate(gs):
                vmap[(w, h, ci)] = gsz * 128
        group_sizes.append(gw)

    per_core = []
    for c in range(NCORES):
        gidx = np.full((TC, 128), -1, dtype=np.int16)
        wsl = np.full((TC, 128), -1.0, dtype=np.float32)   # dst slot in window
        lo, hi = core_bounds[c], core_bounds[c + 1]
        d = dsts[lo:hi] - c * RPC
        s_ids = tab_ids[lo:hi]
        wb = np.searchsorted(d, np.arange(MT + 1) * 128)
        for w in range(MT):
            a, b = wb[w], wb[w + 1]
            hv = s_ids[a:b] >= HB if nh == 2 else np.zeros(b - a, dtype=bool)
            for h in range(nh):
                sel = hv if h else ~hv
                sid = (s_ids[a:b][sel] - h * HB).astype(np.int16)
                slot = (d[a:b][sel] - w * 128).astype(np.float32)
                k = np.arange(sid.size)
                tg = chunk_base[w, h] + (k // 128)
                row = k % 128
                gidx[tg, row] = sid
                wsl[tg, row] = slot
                # equalize live counts across cores: pad with idx 0 (masked
                # by wsl=-1) up to the call's uniform count; the rest stay -1
                off = 0
                for ci, gsz in enumerate(group_sizes[w][h]):
                    p0 = off * 128
                    v = vmap[(w, h, ci)]
                    cnt = int(np.clip(sid.size - p0, 0, gsz * 128))
                    if cnt < v:
                        kk = np.arange(p0 + cnt, p0 + v)
                        gidx[chunk_base[w, h] + (kk // 128), kk % 128] = 0
                    off += gsz
        # wrapped int16 index layout, one block per gather call
        cols = []
        for w in range(MT):
            for h in range(nh):
                t0 = int(chunk_base[w, h])
                for gsz in group_sizes[w][h]:
                    L = gidx[t0:t0 + gsz].reshape(-1)
                    cols.append(np.tile(L.reshape(-1, 16).T, (8, 1)))
                    t0 += gsz
        idx_np = np.ascontiguousarray(np.concatenate(cols, axis=1))
        per_core.append({
            "idx": idx_np,
            "wsl": np.ascontiguousarray(wsl.T),   # [128, TC]
        })
    return {
        "group_sizes": group_sizes, "chunk_base": chunk_base, "TC": TC,
        "vmap": vmap, "per_core": per_core,
    }


def _prepare(edge_index):
    src = np.asarray(edge_index[0], dtype=np.int64)
    dst = np.asarray(edge_index[1], dtype=np.int64)
    order = np.argsort(dst, kind="stable")
    src, dsts = src[order], dst[order]
    core_bounds = np.searchsorted(dsts, np.arange(NCORES + 1) * RPC)
    p1 = _plan(_table_l1(src).astype(np.int64), dsts, core_bounds, 1, NPAD)
    p2 = _plan(_table_l2(src).astype(np.int64), dsts, core_bounds, 2, RA2)
    return p1, p2


def _build(p1, p2):
    # mdt: matmul-operand dtype; tdt: gathered-table dtype; trdt: transpose dtype
    mdt = {"f32": _f32, "f32r": _f32r, "bf16": _bf16, "fp8": _bf16}[MODE]
    tdt = {"f32": _f32, "f32r": _f32, "bf16": _bf16, "fp8": _f8}[MODE]
    trdt = _bf16 if MODE in ("bf16", "fp8") else _f32
    # DoubleRow: two fp8 128-edge chunks per matmul (virtual K=256)
    DR = MODE == "fp8" and os.environ.get("GNN_DR", "1") == "1"
    TC1, TC2 = p1["TC"], p2["TC"]

    nc = bacc.Bacc("TRN2", target_bir_lowering=False, debug=False,
                   num_devices=NCORES, num_swdge_queues=NQ)
    xs_d = nc.dram_tensor("xs", [NPAD, D], tdt, kind="ExternalInput")
    xsk_d = nc.dram_tensor("xsk", [RPAD, D], tdt, kind="ExternalInput")
    dinv_d = nc.dram_tensor("dinv", [RPAD], _f32, kind="ExternalInput")
    W_d = {k: nc.dram_tensor(k, [D, D], mdt, kind="ExternalInput")
           for k in ("W1", "W2", "Wf1")}
    Wf2_d = nc.dram_tensor("Wf2", [D, D_OUT], mdt, kind="ExternalInput")
    b1_d = nc.dram_tensor("b1", [D], _f32, kind="ExternalInput")
    b2r_d = nc.dram_tensor("b2r", [1, D], mdt, kind="ExternalInput")
    sqd_d = nc.dram_tensor("sqd", [1, RPAD], mdt, kind="ExternalInput")
    bf1_d = nc.dram_tensor("bf1", [D], _f32, kind="ExternalInput")
    bf2r_d = nc.dram_tensor("bf2r", [1, D_OUT], mdt, kind="ExternalInput")
    idx1_d = nc.dram_tensor("idx1", [128, TC1 * 8], _i16, kind="ExternalInput")
    wsl1_d = nc.dram_tensor("wsl1", [128, TC1], _f32, kind="ExternalInput")
    idx2_d = nc.dram_tensor("idx2", [128, TC2 * 8], _i16, kind="ExternalInput")
    wsl2_d = nc.dram_tensor("wsl2", [128, TC2], _f32, kind="ExternalInput")
    out_d = nc.dram_tensor("out", [RPAD, D_OUT], _f32, kind="ExternalOutput")

    # L2 AllGather: part A/B as separate tensors so each AllGather's input
    # dependency covers only the phase-A writes that actually feed it
    cc_inA = nc.dram_tensor("cc_inA", [SPLIT2, D], tdt, kind="Internal")
    cc_inB = nc.dram_tensor("cc_inB", [RPAD - SPLIT2, D], tdt, kind="Internal")
    cc_out = nc.dram_tensor("cc_out", [NPAD, D], tdt, kind="Internal",
                            addr_space="Shared")

    RG = [list(range(NCORES))]
    ACT = mybir.ActivationFunctionType
    ALU = mybir.AluOpType

    with tile.TileContext(nc) as tc, ExitStack() as ctx:
        const = ctx.enter_context(tc.tile_pool(name="const", bufs=1))
        actT = ctx.enter_context(tc.tile_pool(name="actT", bufs=2))
        work = ctx.enter_context(tc.tile_pool(name="work", bufs=2))
        aggp = ctx.enter_context(tc.tile_pool(name="aggp", bufs=2))
        msgp = ctx.enter_context(tc.tile_pool(name="msgp", bufs=8))
        sp = ctx.enter_context(tc.tile_pool(name="sp", bufs=8))
        spillp = ctx.enter_context(tc.tile_pool(name="spillp", bufs=MT))
        psA = ctx.enter_context(tc.tile_pool(name="psA", bufs=2, space="PSUM"))
        psC = ctx.enter_context(tc.tile_pool(name="psC", bufs=4, space="PSUM"))
        psT = ctx.enter_context(tc.tile_pool(name="psT", bufs=2, space="PSUM"))

        # ---- constants; L1 gather prerequisites (idx1/wsl1) first ----
        idx1_t = const.tile([128, TC1 * 8], _i16)
        nc.sync.dma_start(idx1_t[:], idx1_d.ap())
        wsl1_t = const.tile([128, TC1], _f32)
        nc.sync.dma_start(wsl1_t[:], wsl1_d.ap())
        dinv_t = const.tile([128, MT], _f32)
        nc.sync.dma_start(dinv_t[:], dinv_d.ap().rearrange("(a p) -> p a", p=128))
        xsk_t = const.tile([128, MT, D], tdt)
        nc.sync.dma_start(xsk_t[:], xsk_d.ap().rearrange("(m p) f -> p m f", p=128))
        w_t = {}
        for k in ("W1", "W2", "Wf1"):
            w_t[k] = const.tile([128, 4, D], mdt, name=f"wt_{k}")
        nc.sync.dma_start(w_t["W1"][:], W_d["W1"].ap().rearrange("(k p) n -> p k n", p=128))
        b1c_t = const.tile([128, 4], _f32)
        nc.sync.dma_start(b1c_t[:], b1_d.ap().rearrange("(a p) -> p a", p=128))

        # the rest on the Activation-engine HWDGE queue (Sync stays free)
        nc.scalar.dma_start(w_t["W2"][:], W_d["W2"].ap().rearrange("(k p) n -> p k n", p=128))
        nc.scalar.dma_start(w_t["Wf1"][:], W_d["Wf1"].ap().rearrange("(k p) n -> p k n", p=128))
        wf2_t = const.tile([128, 4, D_OUT], mdt)
        nc.scalar.dma_start(wf2_t[:], Wf2_d.ap().rearrange("(k p) n -> p k n", p=128))
        b2r_t = const.tile([1, D], mdt)
        nc.scalar.dma_start(b2r_t[:], b2r_d.ap())
        sqd_t = const.tile([1, RPAD], mdt)
        nc.scalar.dma_start(sqd_t[:], sqd_d.ap())
        bf1_t = const.tile([128, 4], _f32)
        nc.scalar.dma_start(bf1_t[:], bf1_d.ap().rearrange("(a p) -> p a", p=128))
        bf2r_t = const.tile([1, D_OUT], mdt)
        nc.scalar.dma_start(bf2r_t[:], bf2r_d.ap())
        idx2_t = const.tile([128, TC2 * 8], _i16)
        nc.scalar.dma_start(idx2_t[:], idx2_d.ap())
        wsl2_t = const.tile([128, TC2], _f32)
        nc.scalar.dma_start(wsl2_t[:], wsl2_d.ap())

        ones_t = const.tile([1, 128], mdt)
        nc.vector.memset(ones_t[:], 1.0)
        hsk2_t = const.tile([128, MT, D], tdt, name="hsk2")
        ident = const.tile([128, 128], trdt)
        make_identity(nc, ident[:])
        iota_i = const.tile([128, G, 128], _i32)
        nc.gpsimd.iota(iota_i[:], pattern=[[0, G], [1, 128]], base=0, channel_multiplier=0)
        iota_h = const.tile([128, G, 128], _f32)
        nc.vector.tensor_copy(iota_h[:], iota_i[:])

        qload = [0] * NQ

        def gather_half(plan, idx_t, wsl_t, src_ap, w, h, ps, tail_mm):
            """Gathers + one-hot chunk matmuls for window w, region h."""
            group_sizes, chunk_base, vmap = (
                plan["group_sizes"], plan["chunk_base"], plan["vmap"])
            t0 = int(chunk_base[w][h])
            col0 = t0 * 8
            if DR:
                n_chunk = sum((g // 2) + (g % 2) for g in group_sizes[w][h])
            else:
                n_chunk = sum(group_sizes[w][h])
            n = n_chunk + len(tail_mm)
            done = 0
            for ci, gsz in enumerate(group_sizes[w][h]):
                msg = msgp.tile([128, G, D], tdt, tag="msg")
                q = min(range(NQ), key=lambda i: qload[i])
                qload[q] += vmap[(w, h, ci)]
                # single_packet=False keeps gather packets small so the SDMA
                # engines round-robin fairly with the collective's rings --
                # with whole-call packets a concurrent AllGather is starved
                # (measured 75us vs ~11us idle); gather rate is unaffected
                nc.gpsimd.dma_gather(msg[:, :gsz, :], src_ap,
                                     idx_t[:, col0:col0 + gsz * 8],
                                     gsz * 128, vmap[(w, h, ci)], D,
                                     queue_num=q, single_packet=False)
                s_t = sp.tile([128, G, 128], tdt, tag="S")
                nc.vector.tensor_tensor(
                    s_t[:, :gsz, :], iota_h[:, :gsz, :],
                    wsl_t[:, t0:t0 + gsz].to_broadcast([128, gsz, 128]),
                    op=ALU.is_equal)
                t = 0
                while t < gsz:
                    if DR and t + 1 < gsz:
                        nc.tensor.matmul(
                            ps[:], lhsT=s_t[:, t:t + 2, :],
                            rhs=msg[:, t:t + 2, :],
                            start=(done == 0), stop=(done == n - 1),
                            perf_mode=mybir.MatmulPerfMode.DoubleRow)
                        t += 2
                    else:
                        nc.tensor.matmul(ps[:], lhsT=s_t[:, t, :],
                                         rhs=msg[:, t, :],
                                         start=(done == 0), stop=(done == n - 1))
                        t += 1
                    done += 1
                t0 += gsz
                col0 += gsz * 8
            for lhsT, rhs in tail_mm:
                nc.tensor.matmul(ps[:], lhsT=lhsT, rhs=rhs,
                                 start=False, stop=(done == n - 1))
                done += 1

        # ================= layer 1: gather-then-dense, no AllGather ========
        z1T = actT.tile([128, 4, RPAD], mdt, tag="zT")
        src1 = xs_d.ap().bitcast(tdt)

        def l1_dense(w, aggdT):
            # z1T blocks for window w: relu(W1.T-block @ aggdT + b1)
            for q in range(4):
                pd = psA.tile([128, 128], _f32, tag="psA", name="pd")
                for k in range(4):
                    nc.tensor.matmul(pd[:],
                                     lhsT=w_t["W1"][:, k, q * 128:(q + 1) * 128],
                                     rhs=aggdT[:, k, :],
                                     start=(k == 0), stop=(k == 3))
                nc.scalar.activation(z1T[:, q, w * 128:(w + 1) * 128], pd[:],
                                     ACT.Relu, bias=b1c_t[:, q:q + 1])
            # phase A of layer 2 for this m-tile: hs2 = dinv * (z1 @ W2)
            ps = psA.tile([128, D], _f32, tag="psA")
            for k in range(4):
                nc.tensor.matmul(ps[:], lhsT=z1T[:, k, w * 128:(w + 1) * 128],
                                 rhs=w_t["W2"][:, k, :], start=(k == 0), stop=(k == 3))
            nc.scalar.activation(hsk2_t[:, w, :], ps[:], ACT.Copy,
                                 scale=dinv_t[:, w:w + 1])
            part, r0 = (cc_inA, 0) if w < MA2 else (cc_inB, SPLIT2)
            nc.scalar.dma_start(
                part.ap()[w * 128 - r0:(w + 1) * 128 - r0, :], hsk2_t[:, w, :])
            if w == MA2 - 1:
                nc.gpsimd.collective_compute(
                    "AllGather", ALU.bypass, ins=[cc_inA.ap()],
                    outs=[cc_out.ap()[0:RA2]], replica_groups=RG)
            elif w == MT - 1:
                nc.gpsimd.collective_compute(
                    "AllGather", ALU.bypass, ins=[cc_inB.ap()],
                    outs=[cc_out.ap()[RA2:NPAD]], replica_groups=RG)

        pending = None  # (w, aggdT): dense deferred one window for overlap
        for w in range(MT):
            ps = psC.tile([128, D], _f32, tag="psC", name=f"agg_{w}")
            gather_half(p1, idx1_t, wsl1_t, src1, w, 0, ps,
                        [(ident[:], xsk_t[:, w, :])])
            # dinv-scale on the PSUM->SBUF copy (commutes with @W1)
            aggsb = work.tile([128, D], trdt, tag="aggsb")
            nc.scalar.activation(aggsb[:], ps[:], ACT.Copy,
                                 scale=dinv_t[:, w:w + 1])
            aggdT = aggp.tile([128, 4, 128], trdt, tag="aggdT")
            # hardware X-bar transpose straight into the blocked layout:
            # aggdT[p, q, s] = aggsb[s, q*128+p] -- no PE/Scalar involvement
            nc.sync.dma_start_transpose(aggdT[:], aggsb[:])
            if pending is not None:
                l1_dense(*pending)
            pending = (w, aggdT)
        l1_dense(*pending)

        # ================= layer 2 aggregation + head ======================
        z2T = actT.tile([128, 4, RPAD], mdt, tag="zT")
        z3T = actT.tile([128, 4, RPAD], mdt, tag="zT3")

        def head_window(m):
            for q in range(4):
                ps = psA.tile([128, D], _f32, tag="psA")
                for k in range(4):
                    nc.tensor.matmul(ps[:, 0:128],
                                     lhsT=w_t["Wf1"][:, k, q * 128:(q + 1) * 128],
                                     rhs=z2T[:, k, m * 128:(m + 1) * 128],
                                     start=(k == 0), stop=(k == 3))
                nc.scalar.activation(z3T[:, q, m * 128:(m + 1) * 128], ps[:, 0:128],
                                     ACT.Relu, bias=bf1_t[:, q:q + 1])
            ps2 = psT.tile([128, D_OUT], _f32, tag="psT")
            for k in range(4):
                nc.tensor.matmul(ps2[:], lhsT=z3T[:, k, m * 128:(m + 1) * 128],
                                 rhs=wf2_t[:, k, :], start=(k == 0), stop=False)
            nc.tensor.matmul(ps2[:], lhsT=ones_t[0:1, :], rhs=bf2r_t[0:1, :],
                             start=False, stop=True)
            nmx = work.tile([128, 1], _f32, tag="nmx")
            nc.vector.tensor_reduce(nmx[:], ps2[:], axis=mybir.AxisListType.X,
                                    op=ALU.max, negate=True)
            ex = work.tile([128, D_OUT], _f32, tag="ex")
            sm = work.tile([128, 1], _f32, tag="sm")
            nc.scalar.activation(ex[:], ps2[:], ACT.Exp, bias=nmx[:, :1], scale=1.0,
                                 accum_out=sm[:, :1])
            rin = work.tile([128, 1], _f32, tag="rin")
            nc.vector.reciprocal(rin[:], sm[:])
            ot = work.tile([128, D_OUT], _f32, tag="ot")
            nc.vector.tensor_tensor(ot[:], ex[:], rin[:, :1].to_broadcast([128, D_OUT]),
                                    op=ALU.mult)
            nc.sync.dma_start(out_d.ap()[m * 128:(m + 1) * 128, :], ot[:])

        spills = {}
        srcA = cc_out.ap()[0:RA2].bitcast(tdt)
        srcB = cc_out.ap()[RA2:NPAD].bitcast(tdt)
        for w in range(MT):
            ps = psC.tile([128, D], _f32, tag="psC", name=f"ps1_{w}")
            gather_half(p2, idx2_t, wsl2_t, srcA, w, 0, ps,
                        [(ident[:], hsk2_t[:, w, :])])
            sp_w = spillp.tile([128, D], trdt, tag="spill", name=f"spill_{w}")
            nc.scalar.copy(sp_w[:], ps[:])
            spills[w] = sp_w

        for w in range(MT):
            ps = psC.tile([128, D], _f32, tag="psC", name=f"ps2_{w}")
            gather_half(p2, idx2_t, wsl2_t, srcB, w, 1, ps, [
                (ident[:], spills[w][:]),
                (sqd_t[0:1, w * 128:(w + 1) * 128], b2r_t[0:1, :]),
            ])
            zrel = work.tile([128, D], trdt, tag="zrel")
            nc.scalar.activation(zrel[:], ps[:], ACT.Relu,
                                 scale=dinv_t[:, w:w + 1])
            nc.sync.dma_start_transpose(z2T[:, :, w * 128:(w + 1) * 128], zrel[:])
            head_window(w)

    nc.compile()
    return nc


def _run(inputs, trace=False):
    x = np.asarray(inputs["x"], dtype=np.float32)
    edge_index = np.asarray(inputs["edge_index"])
    deg = np.bincount(
        np.concatenate([edge_index[1], np.arange(N, dtype=edge_index.dtype)]),
        minlength=N,
    ).astype(np.float32)
    dinv = np.zeros(N, dtype=np.float32)
    nz = deg > 0
    dinv[nz] = (1.0 / np.sqrt(deg[nz])).astype(np.float32)
    sqd = np.zeros(N, dtype=np.float32)
    sqd[nz] = np.sqrt(deg[nz]).astype(np.float32)

    p1, p2 = _prepare(edge_index)
    nc = _build(p1, p2)

    import ml_dtypes
    if MODE in ("bf16", "fp8"):
        mnp = ml_dtypes.bfloat16
    else:
        mnp = np.float32
    tnp = {"f32": np.float32, "f32r": np.float32, "bf16": ml_dtypes.bfloat16,
           "fp8": ml_dtypes.float8_e4m3}[MODE]

    # full dinv-scaled x table in the padded core-major layout
    xs_full = np.zeros((NPAD, D), dtype=np.float32)
    for c in range(NCORES):
        xs_full[c * RPAD:c * RPAD + RPC] = (
            dinv[c * RPC:(c + 1) * RPC, None] * x[c * RPC:(c + 1) * RPC])
    xs_full = np.ascontiguousarray(xs_full).astype(tnp)

    in_maps = []
    for c in range(NCORES):
        dv = np.zeros(RPAD, dtype=np.float32)
        dv[:RPC] = dinv[c * RPC:(c + 1) * RPC]
        sq = np.zeros(RPAD, dtype=np.float32)
        sq[:RPC] = sqd[c * RPC:(c + 1) * RPC]
        in_maps.append({
            "xs": xs_full,
            "xsk": np.ascontiguousarray(xs_full[c * RPAD:(c + 1) * RPAD]),
            "dinv": dv,
            "sqd": sq.reshape(1, RPAD).astype(mnp),
            "W1": np.asarray(inputs["W1"], np.float32).astype(mnp),
            "W2": np.asarray(inputs["W2"], np.float32).astype(mnp),
            "Wf1": np.asarray(inputs["Wf1"], np.float32).astype(mnp),
            "Wf2": np.asarray(inputs["Wf2"], np.float32).astype(mnp),
            "b1": np.asarray(inputs["b1"], np.float32),
            "b2r": np.asarray(inputs["b2"], np.float32).reshape(1, D).astype(mnp),
            "bf1": np.asarray(inputs["bf1"], np.float32),
            "bf2r": np.asarray(inputs["bf2"], np.float32).reshape(1, D_OUT).astype(mnp),
            "idx1": p1["per_core"][c]["idx"],
            "wsl1": p1["per_core"][c]["wsl"],
            "idx2": p2["per_core"][c]["idx"],
            "wsl2": p2["per_core"][c]["wsl"],
        })

    res = run_bass_kernel_spmd(nc, in_maps, core_ids=list(range(NCORES)),
                               trace=trace)
    out = np.concatenate([res.results[c]["out"][:RPC] for c in range(NCORES)], axis=0)
    return out, res


def kernel(**inputs):
    out, _ = _run(inputs, trace=False)
    return out


# revision 38
# speedup vs baseline: 1.7226x; 1.7226x over previous
"""GCN (2x GCNConv + MLP head + softmax) on 8 TRN2 NeuronCores.

Strategy (graph/data parallel, per sharding hint):
  - Nodes sharded across 8 cores (2500 rows each, padded to 2560); weights
    replicated. Edges partitioned by dst; aggregation runs per dst window
    (128 slots) as dma_gather row gathers (4 SWDGE queues; the gather
    stream is the roofline resource at ~43 GB/s/queue) + one-hot segment
    matmuls on the TensorEngine accumulating in PSUM; fp8 chunk pairs run
    as DoubleRow matmuls (virtual K=256). One-hot matrices are built
    on-chip (DVE iota==slot); self-loops never gather (identity matmul of
    resident rows); trailing pad slots carry idx=-1 so the gather ucode
    skips their descriptors (num_idxs_reg = per-call live count, equalized
    across cores so the SPMD program stays uniform).
  - Layer 1 needs NO AllGather: aggregation commutes with the dense matmul
    (Ahat @ (x W1) == (Ahat @ x) W1), so the host stages the full
    dinv-scaled x as an fp8 table and gathers start immediately. Per
    window: aggregate -> dinv-scale on the PSUM->SBUF copy -> transpose ->
    dense @W1 emitting z1T blocks directly (bias+relu on the Scalar
    engine), software-pipelined one window behind the aggregation.
  - Layer 2 AllGathers the dinv-scaled h2 table in two parts (A=2048
    rows/core fired after z1 window 15, B=512 after window 19) so ~80% of
    its gathers never wait on the last z1 windows. Pass 1 aggregates
    region-A sources for all windows (spilling partials to SBUF), pass 2
    re-adds the spill via identity matmul, folds the bias in as a rank-1
    (sqrt(deg) x bias) matmul, and fuses the final dinv scaling into the
    Scalar-engine relu.
  - Head: two dense layers + row softmax per 128-row window of layer 2.

Host-side preprocessing is limited to graph-structure work (edge sort,
degree counts, window slots, gather-index layout, dinv*x staging).
"""

import os
from contextlib import ExitStack

import numpy as np

import concourse.bacc as bacc
import concourse.mybir as mybir
import concourse.tile as tile
from concourse.bass_utils import run_bass_kernel_spmd
from concourse.masks import make_identity

# problem shapes (hardcoded per contract)
N = 20000
E = 320000
D = 512
D_OUT = 128
NCORES = 8
RPC = 2500          # real rows per core
RPAD = 2560         # padded rows per core (20 tiles of 128)
NPAD = RPAD * NCORES
MT = RPAD // 128    # m-tiles / dst windows per core (20)
SPLIT2 = int(os.environ.get("GNN_SPLIT2", "1280"))  # L2 AG part-A rows/core
RA2 = SPLIT2 * NCORES
MA2 = SPLIT2 // 128
G = 6               # max chunks (of 128 edges) per dma_gather call
NQ = 4              # SWDGE queues for gather rotation

# config: "f32" (exact), "f32r" (fast fp32 matmul), "bf16" (half-traffic),
# "fp8" (bf16 compute, fp8e4 gathered tables: quarter gather/AG traffic)
MODE = os.environ.get("GNN_MODE", "fp8")

_f32 = mybir.dt.float32
_f32r = mybir.dt.float32r
_bf16 = mybir.dt.bfloat16
_f8 = mybir.dt.float8e4
_i16 = mybir.dt.int16
_i32 = mybir.dt.int32


def _table_l1(node):
    """Row of node in the host-staged xs table (padded core-major layout)."""
    return (node // RPC) * RPAD + (node % RPC)


def _table_l2(node):
    """Row of node in the L2 AllGather-ed table (part A then part B)."""
    c, r = node // RPC, node % RPC
    h = r >= SPLIT2
    return np.where(h, RA2 + c * (RPAD - SPLIT2) + (r - SPLIT2), c * SPLIT2 + r)


def _plan(tab_ids, dsts, core_bounds, nh, HB):
    """Window/chunk/call plan for one layer's gathers.

    tab_ids: per-edge table row (edges sorted by dst); nh: number of table
    regions (1 for L1, 2 for L2); HB: region-A size (region boundary).
    Returns dict with group_sizes[w][h], chunk_base[w][h], TC,
    vmap[(w,h,ci)], and per-core idx/wsl arrays.
    """
    counts = np.zeros((NCORES, MT, nh), dtype=np.int64)
    for c in range(NCORES):
        lo, hi = core_bounds[c], core_bounds[c + 1]
        d = dsts[lo:hi] - c * RPC
        hvec = (tab_ids[lo:hi] >= HB).astype(np.int64) if nh == 2 else None
        wb = np.searchsorted(d, np.arange(MT + 1) * 128)
        for w in range(MT):
            a, b = wb[w], wb[w + 1]
            if nh == 2:
                n1 = int(hvec[a:b].sum())
                counts[c, w, 0] = (b - a) - n1
                counts[c, w, 1] = n1
            else:
                counts[c, w, 0] = b - a

    cpw = np.maximum(1, -(-counts.max(axis=0) // 128))  # [MT, nh] chunks
    TC = int(cpw.sum())
    cb = np.concatenate([[0], np.cumsum(cpw.reshape(-1))]).astype(int)
    chunk_base = cb[:-1].reshape(MT, nh)

    group_sizes = []
    vmap = {}
    for w in range(MT):
        gw = []
        for h in range(nh):
            n = int(cpw[w, h])
            k = -(-n // G)
            base, rem = divmod(n, k)
            gs = [base + (i < rem) for i in range(k)]
            gw.append(gs)
            # gather every slot (pads hit row 0, masked by wsl=-1): ~4%
            # extra rows, but msg buffers then never hold stale bytes, so
            # the startup memset pass is unnecessary
            for ci, gsz in enumerate(gs):
                vmap[(w, h, ci)] = gsz * 128
        group_sizes.append(gw)

    per_core = []
    for c in range(NCORES):
        gidx = np.full((TC, 128), -1, dtype=np.int16)
        wsl = np.full((TC, 128), -1.0, dtype=np.float32)   # dst slot in window
        lo, hi = core_bounds[c], core_bounds[c + 1]
        d = dsts[lo:hi] - c * RPC
        s_ids = tab_ids[lo:hi]
        wb = np.searchsorted(d, np.arange(MT + 1) * 128)
        for w in range(MT):
            a, b = wb[w], wb[w + 1]
            hv = s_ids[a:b] >= HB if nh == 2 else np.zeros(b - a, dtype=bool)
            for h in range(nh):
                sel = hv if h else ~hv
                sid = (s_ids[a:b][sel] - h * HB).astype(np.int16)
                slot = (d[a:b][sel] - w * 128).astype(np.float32)
                k = np.arange(sid.size)
                tg = chunk_base[w, h] + (k // 128)
                row = k % 128
                gidx[tg, row] = sid
                wsl[tg, row] = slot
                # equalize live counts across cores: pad with idx 0 (masked
                # by wsl=-1) up to the call's uniform count; the rest stay -1
                off = 0
                for ci, gsz in enumerate(group_sizes[w][h]):
                    p0 = off * 128
                    v = vmap[(w, h, ci)]
                    cnt = int(np.clip(sid.size - p0, 0, gsz * 128))
                    if cnt < v:
                        kk = np.arange(p0 + cnt, p0 + v)
                        gidx[chunk_base[w, h] + (kk // 128), kk % 128] = 0
                    off += gsz
        # wrapped int16 index layout, one block per gather call
        cols = []
        for w in range(MT):
            for h in range(nh):
                t0 = int(chunk_base[w, h])
                for gsz in group_sizes[w][h]:
                    L = gidx[t0:t0 + gsz].reshape(-1)
                    cols.append(np.tile(L.reshape(-1, 16).T, (8, 1)))
                    t0 += gsz
        idx_np = np.ascontiguousarray(np.concatenate(cols, axis=1))
        per_core.append({
            "idx": idx_np,
            "wsl": np.ascontiguousarray(wsl.T),   # [128, TC]
        })
    return {
        "group_sizes": group_sizes, "chunk_base": chunk_base, "TC": TC,
        "vmap": vmap, "per_core": per_core,
    }


def _prepare(edge_index):
    src = np.asarray(edge_index[0], dtype=np.int64)
    dst = np.asarray(edge_index[1], dtype=np.int64)
    order = np.argsort(dst, kind="stable")
    src, dsts = src[order], dst[order]
    core_bounds = np.searchsorted(dsts, np.arange(NCORES + 1) * RPC)
    p1 = _plan(_table_l1(src).astype(np.int64), dsts, core_bounds, 1, NPAD)
    p2 = _plan(_table_l2(src).astype(np.int64), dsts, core_bounds, 2, RA2)
    return p1, p2


def _build(p1, p2):
    # mdt: matmul-operand dtype; tdt: gathered-table dtype; trdt: transpose dtype
    mdt = {"f32": _f32, "f32r": _f32r, "bf16": _bf16, "fp8": _bf16}[MODE]
    tdt = {"f32": _f32, "f32r": _f32, "bf16": _bf16, "fp8": _f8}[MODE]
    trdt = _bf16 if MODE in ("bf16", "fp8") else _f32
    # DoubleRow: two fp8 128-edge chunks per matmul (virtual K=256)
    DR = MODE == "fp8" and os.environ.get("GNN_DR", "1") == "1"
    TC1, TC2 = p1["TC"], p2["TC"]

    nc = bacc.Bacc("TRN2", target_bir_lowering=False, debug=False,
                   num_devices=NCORES, num_swdge_queues=NQ)
    xs_d = nc.dram_tensor("xs", [NPAD, D], tdt, kind="ExternalInput")
    xsk_d = nc.dram_tensor("xsk", [RPAD, D], tdt, kind="ExternalInput")
    dinv_d = nc.dram_tensor("dinv", [RPAD], _f32, kind="ExternalInput")
    W_d = {k: nc.dram_tensor(k, [D, D], mdt, kind="ExternalInput")
           for k in ("W1", "W2", "Wf1")}
    Wf2_d = nc.dram_tensor("Wf2", [D, D_OUT], mdt, kind="ExternalInput")
    b1_d = nc.dram_tensor("b1", [D], _f32, kind="ExternalInput")
    b2r_d = nc.dram_tensor("b2r", [1, D], mdt, kind="ExternalInput")
    sqd_d = nc.dram_tensor("sqd", [1, RPAD], mdt, kind="ExternalInput")
    bf1_d = nc.dram_tensor("bf1", [D], _f32, kind="ExternalInput")
    bf2r_d = nc.dram_tensor("bf2r", [1, D_OUT], mdt, kind="ExternalInput")
    idx1_d = nc.dram_tensor("idx1", [128, TC1 * 8], _i16, kind="ExternalInput")
    wsl1_d = nc.dram_tensor("wsl1", [128, TC1], _f32, kind="ExternalInput")
    idx2_d = nc.dram_tensor("idx2", [128, TC2 * 8], _i16, kind="ExternalInput")
    wsl2_d = nc.dram_tensor("wsl2", [128, TC2], _f32, kind="ExternalInput")
    out_d = nc.dram_tensor("out", [RPAD, D_OUT], _f32, kind="ExternalOutput")

    # L2 AllGather: part A/B as separate tensors so each AllGather's input
    # dependency covers only the phase-A writes that actually feed it
    cc_inA = nc.dram_tensor("cc_inA", [SPLIT2, D], tdt, kind="Internal")
    cc_inB = nc.dram_tensor("cc_inB", [RPAD - SPLIT2, D], tdt, kind="Internal")
    cc_out = nc.dram_tensor("cc_out", [NPAD, D], tdt, kind="Internal",
                            addr_space="Shared")

    RG = [list(range(NCORES))]
    ACT = mybir.ActivationFunctionType
    ALU = mybir.AluOpType

    with tile.TileContext(nc) as tc, ExitStack() as ctx:
        const = ctx.enter_context(tc.tile_pool(name="const", bufs=1))
        actT = ctx.enter_context(tc.tile_pool(name="actT", bufs=2))
        work = ctx.enter_context(tc.tile_pool(name="work", bufs=2))
        aggp = ctx.enter_context(tc.tile_pool(name="aggp", bufs=2))
        msgp = ctx.enter_context(tc.tile_pool(name="msgp", bufs=8))
        sp = ctx.enter_context(tc.tile_pool(name="sp", bufs=8))
        spillp = ctx.enter_context(tc.tile_pool(name="spillp", bufs=MT))
        psA = ctx.enter_context(tc.tile_pool(name="psA", bufs=2, space="PSUM"))
        psC = ctx.enter_context(tc.tile_pool(name="psC", bufs=4, space="PSUM"))
        psT = ctx.enter_context(tc.tile_pool(name="psT", bufs=2, space="PSUM"))

        # ---- constants; L1 gather prerequisites (idx1/wsl1) first ----
        idx1_t = const.tile([128, TC1 * 8], _i16)
        nc.sync.dma_start(idx1_t[:], idx1_d.ap())
        wsl1_t = const.tile([128, TC1], _f32)
        nc.sync.dma_start(wsl1_t[:], wsl1_d.ap())
        dinv_t = const.tile([128, MT], _f32)
        nc.sync.dma_start(dinv_t[:], dinv_d.ap().rearrange("(a p) -> p a", p=128))
        xsk_t = const.tile([128, MT, D], tdt)
        nc.sync.dma_start(xsk_t[:], xsk_d.ap().rearrange("(m p) f -> p m f", p=128))
        w_t = {}
        for k in ("W1", "W2", "Wf1"):
            w_t[k] = const.tile([128, 4, D], mdt, name=f"wt_{k}")
        nc.sync.dma_start(w_t["W1"][:], W_d["W1"].ap().rearrange("(k p) n -> p k n", p=128))
        b1c_t = const.tile([128, 4], _f32)
        nc.sync.dma_start(b1c_t[:], b1_d.ap().rearrange("(a p) -> p a", p=128))

        # the rest on the Activation-engine HWDGE queue (Sync stays free)
        nc.scalar.dma_start(w_t["W2"][:], W_d["W2"].ap().rearrange("(k p) n -> p k n", p=128))
        nc.scalar.dma_start(w_t["Wf1"][:], W_d["Wf1"].ap().rearrange("(k p) n -> p k n", p=128))
        wf2_t = const.tile([128, 4, D_OUT], mdt)
        nc.scalar.dma_start(wf2_t[:], Wf2_d.ap().rearrange("(k p) n -> p k n", p=128))
        b2r_t = const.tile([1, D], mdt)
        nc.scalar.dma_start(b2r_t[:], b2r_d.ap())
        sqd_t = const.tile([1, RPAD], mdt)
        nc.scalar.dma_start(sqd_t[:], sqd_d.ap())
        bf1_t = const.tile([128, 4], _f32)
        nc.scalar.dma_start(bf1_t[:], bf1_d.ap().rearrange("(a p) -> p a", p=128))
        bf2r_t = const.tile([1, D_OUT], mdt)
        nc.scalar.dma_start(bf2r_t[:], bf2r_d.ap())
        idx2_t = const.tile([128, TC2 * 8], _i16)
        nc.scalar.dma_start(idx2_t[:], idx2_d.ap())
        wsl2_t = const.tile([128, TC2], _f32)
        nc.scalar.dma_start(wsl2_t[:], wsl2_d.ap())

        ones_t = const.tile([1, 128], mdt)
        nc.vector.memset(ones_t[:], 1.0)
        hsk2_t = const.tile([128, MT, D], tdt, name="hsk2")
        ident = const.tile([128, 128], trdt)
        make_identity(nc, ident[:])
        iota_i = const.tile([128, G, 128], _i32)
        nc.gpsimd.iota(iota_i[:], pattern=[[0, G], [1, 128]], base=0, channel_multiplier=0)
        iota_h = const.tile([128, G, 128], _f32)
        nc.vector.tensor_copy(iota_h[:], iota_i[:])

        qload = [0] * NQ

        def gather_half(plan, idx_t, wsl_t, src_ap, w, h, ps, tail_mm):
            """Gathers + one-hot chunk matmuls for window w, region h."""
            group_sizes, chunk_base, vmap = (
                plan["group_sizes"], plan["chunk_base"], plan["vmap"])
            t0 = int(chunk_base[w][h])
            col0 = t0 * 8
            if DR:
                n_chunk = sum((g // 2) + (g % 2) for g in group_sizes[w][h])
            else:
                n_chunk = sum(group_sizes[w][h])
            n = n_chunk + len(tail_mm)
            done = 0
            for ci, gsz in enumerate(group_sizes[w][h]):
                msg = msgp.tile([128, G, D], tdt, tag="msg")
                q = min(range(NQ), key=lambda i: qload[i])
                qload[q] += vmap[(w, h, ci)]
                # single_packet=False keeps gather packets small so the SDMA
                # engines round-robin fairly with the collective's rings --
                # with whole-call packets a concurrent AllGather is starved
                # (measured 75us vs ~11us idle); gather rate is unaffected
                nc.gpsimd.dma_gather(msg[:, :gsz, :], src_ap,
                                     idx_t[:, col0:col0 + gsz * 8],
                                     gsz * 128, vmap[(w, h, ci)], D,
                                     queue_num=q, single_packet=False)
                s_t = sp.tile([128, G, 128], tdt, tag="S")
                nc.vector.tensor_tensor(
                    s_t[:, :gsz, :], iota_h[:, :gsz, :],
                    wsl_t[:, t0:t0 + gsz].to_broadcast([128, gsz, 128]),
                    op=ALU.is_equal)
                t = 0
                while t < gsz:
                    if DR and t + 1 < gsz:
                        nc.tensor.matmul(
                            ps[:], lhsT=s_t[:, t:t + 2, :],
                            rhs=msg[:, t:t + 2, :],
                            start=(done == 0), stop=(done == n - 1),
                            perf_mode=mybir.MatmulPerfMode.DoubleRow)
                        t += 2
                    else:
                        nc.tensor.matmul(ps[:], lhsT=s_t[:, t, :],
                                         rhs=msg[:, t, :],
                                         start=(done == 0), stop=(done == n - 1))
                        t += 1
                    done += 1
                t0 += gsz
                col0 += gsz * 8
            for lhsT, rhs in tail_mm:
                nc.tensor.matmul(ps[:], lhsT=lhsT, rhs=rhs,
                                 start=False, stop=(done == n - 1))
                done += 1

        # ================= layer 1: gather-then-dense, no AllGather ========
        z1T = actT.tile([128, 4, RPAD], mdt, tag="zT")
        src1 = xs_d.ap().bitcast(tdt)

        def l1_dense(w, aggdT):
            # z1T blocks for window w: relu(W1.T-block @ aggdT + b1)
            for q in range(4):
                pd = psA.tile([128, 128], _f32, tag="psA", name="pd")
                for k in range(4):
                    nc.tensor.matmul(pd[:],
                                     lhsT=w_t["W1"][:, k, q * 128:(q + 1) * 128],
                                     rhs=aggdT[:, k, :],
                                     start=(k == 0), stop=(k == 3))
                nc.scalar.activation(z1T[:, q, w * 128:(w + 1) * 128], pd[:],
                                     ACT.Relu, bias=b1c_t[:, q:q + 1])
            # phase A of layer 2 for this m-tile: hs2 = dinv * (z1 @ W2)
            ps = psA.tile([128, D], _f32, tag="psA")
            for k in range(4):
                nc.tensor.matmul(ps[:], lhsT=z1T[:, k, w * 128:(w + 1) * 128],
                                 rhs=w_t["W2"][:, k, :], start=(k == 0), stop=(k == 3))
            nc.scalar.activation(hsk2_t[:, w, :], ps[:], ACT.Copy,
                                 scale=dinv_t[:, w:w + 1])
            part, r0 = (cc_inA, 0) if w < MA2 else (cc_inB, SPLIT2)
            nc.scalar.dma_start(
                part.ap()[w * 128 - r0:(w + 1) * 128 - r0, :], hsk2_t[:, w, :])
            if w == MA2 - 1:
                nc.gpsimd.collective_compute(
                    "AllGather", ALU.bypass, ins=[cc_inA.ap()],
                    outs=[cc_out.ap()[0:RA2]], replica_groups=RG)
            elif w == MT - 1:
                nc.gpsimd.collective_compute(
                    "AllGather", ALU.bypass, ins=[cc_inB.ap()],
                    outs=[cc_out.ap()[RA2:NPAD]], replica_groups=RG)

        pending = None  # (w, aggdT): dense deferred one window for overlap
        for w in range(MT):
            ps = psC.tile([128, D], _f32, tag="psC", name=f"agg_{w}")
            gather_half(p1, idx1_t, wsl1_t, src1, w, 0, ps,
                        [(ident[:], xsk_t[:, w, :])])
            # dinv-scale on the PSUM->SBUF copy (commutes with @W1)
            aggsb = work.tile([128, D], trdt, tag="aggsb")
            nc.scalar.activation(aggsb[:], ps[:], ACT.Copy,
                                 scale=dinv_t[:, w:w + 1])
            aggdT = aggp.tile([128, 4, 128], trdt, tag="aggdT")
            for q in range(4):
                pt = psT.tile([128, 128], trdt, tag="psT")
                nc.tensor.transpose(pt[:], aggsb[:, q * 128:(q + 1) * 128], ident[:])
                nc.vector.tensor_copy(aggdT[:, q, :], pt[:])
            if pending is not None:
                l1_dense(*pending)
            pending = (w, aggdT)
        l1_dense(*pending)

        # ================= layer 2 aggregation + head ======================
        z2T = actT.tile([128, 4, RPAD], mdt, tag="zT")
        z3T = actT.tile([128, 4, RPAD], mdt, tag="zT3")

        def head_window(m):
            for q in range(4):
                ps = psA.tile([128, D], _f32, tag="psA")
                for k in range(4):
                    nc.tensor.matmul(ps[:, 0:128],
                                     lhsT=w_t["Wf1"][:, k, q * 128:(q + 1) * 128],
                                     rhs=z2T[:, k, m * 128:(m + 1) * 128],
                                     start=(k == 0), stop=(k == 3))
                nc.scalar.activation(z3T[:, q, m * 128:(m + 1) * 128], ps[:, 0:128],
                                     ACT.Relu, bias=bf1_t[:, q:q + 1])
            ps2 = psT.tile([128, D_OUT], _f32, tag="psT")
            for k in range(4):
                nc.tensor.matmul(ps2[:], lhsT=z3T[:, k, m * 128:(m + 1) * 128],
                                 rhs=wf2_t[:, k, :], start=(k == 0), stop=False)
            nc.tensor.matmul(ps2[:], lhsT=ones_t[0:1, :], rhs=bf2r_t[0:1, :],
                             start=False, stop=True)
            nmx = work.tile([128, 1], _f32, tag="nmx")
            nc.vector.tensor_reduce(nmx[:], ps2[:], axis=mybir.AxisListType.X,
                                    op=ALU.max, negate=True)
            ex = work.tile([128, D_OUT], _f32, tag="ex")
            sm = work.tile([128, 1], _f32, tag="sm")
            nc.scalar.activation(ex[:], ps2[:], ACT.Exp, bias=nmx[:, :1], scale=1.0,
                                 accum_out=sm[:, :1])
            rin = work.tile([128, 1], _f32, tag="rin")
            nc.vector.reciprocal(rin[:], sm[:])
            ot = work.tile([128, D_OUT], _f32, tag="ot")
            nc.vector.tensor_tensor(ot[:], ex[:], rin[:, :1].to_broadcast([128, D_OUT]),
                                    op=ALU.mult)
            nc.sync.dma_start(out_d.ap()[m * 128:(m + 1) * 128, :], ot[:])

        spills = {}
        srcA = cc_out.ap()[0:RA2].bitcast(tdt)
        srcB = cc_out.ap()[RA2:NPAD].bitcast(tdt)
        for w in range(MT):
            ps = psC.tile([128, D], _f32, tag="psC", name=f"ps1_{w}")
            gather_half(p2, idx2_t, wsl2_t, srcA, w, 0, ps,
                        [(ident[:], hsk2_t[:, w, :])])
            sp_w = spillp.tile([128, D], trdt, tag="spill", name=f"spill_{w}")
            nc.scalar.copy(sp_w[:], ps[:])
            spills[w] = sp_w

        for w in range(MT):
            ps = psC.tile([128, D], _f32, tag="psC", name=f"ps2_{w}")
            gather_half(p2, idx2_t, wsl2_t, srcB, w, 1, ps, [
                (ident[:], spills[w][:]),
                (sqd_t[0:1, w * 128:(w + 1) * 128], b2r_t[0:1, :]),
            ])
            zrel = work.tile([128, D], trdt, tag="zrel")
            nc.scalar.activation(zrel[:], ps[:], ACT.Relu,
                                 scale=dinv_t[:, w:w + 1])
            for q in range(4):
                pt = psT.tile([128, 128], trdt, tag="psT")
                nc.tensor.transpose(pt[:], zrel[:, q * 128:(q + 1) * 128], ident[:])
                nc.vector.tensor_copy(z2T[:, q, w * 128:(w + 1) * 128], pt[:])
            head_window(w)

    nc.compile()
    return nc


def _run(inputs, trace=False):
    x = np.asarray(inputs["x"], dtype=np.float32)
    edge_index = np.asarray(inputs["edge_index"])
    deg = np.bincount(
        np.concatenate([edge_index[1], np.arange(N, dtype=edge_index.dtype)]),
        minlength=N,
    ).astype(np.float32)
    dinv = np.zeros(N, dtype=np.float32)
    nz = deg > 0
    dinv[nz] = (1.0 / np.sqrt(deg[nz])).astype(np.float32)
    sqd = np.zeros(N, dtype=np.float32)
    sqd[nz] = np.sqrt(deg[nz]).astype(np.float32)

    p1, p2 = _prepare(edge_index)
    nc = _build(p1, p2)

    import ml_dtypes
    if MODE in ("bf16", "fp8"):
        mnp = ml_dtypes.bfloat16
    else:
        mnp = np.float32
    tnp = {"f32": np.float32, "f32r": np.float32, "bf16": ml_dtypes.bfloat16,
           "fp8": ml_dtypes.float8_e4m3}[MODE]

    # full dinv-scaled x table in the padded core-major layout
    xs_full = np.zeros((NPAD, D), dtype=np.float32)
    for c in range(NCORES):
        xs_full[c * RPAD:c * RPAD + RPC] = (
            dinv[c * RPC:(c + 1) * RPC, None] * x[c * RPC:(c + 1) * RPC])
    xs_full = np.ascontiguousarray(xs_full).astype(tnp)

    in_maps = []
    for c in range(NCORES):
        dv = np.zeros(RPAD, dtype=np.float32)
        dv[:RPC] = dinv[c * RPC:(c + 1) * RPC]
        sq = np.zeros(RPAD, dtype=np.float32)
        sq[:RPC] = sqd[c * RPC:(c + 1) * RPC]
        in_maps.append({
            "xs": xs_full,
            "xsk": np.ascontiguousarray(xs_full[c * RPAD:(c + 1) * RPAD]),
            "dinv": dv,
            "sqd": sq.reshape(1, RPAD).astype(mnp),
            "W1": np.asarray(inputs["W1"], np.float32).astype(mnp),
            "W2": np.asarray(inputs["W2"], np.float32).astype(mnp),
            "Wf1": np.asarray(inputs["Wf1"], np.float32).astype(mnp),
            "Wf2": np.asarray(inputs["Wf2"], np.float32).astype(mnp),
            "b1": np.asarray(inputs["b1"], np.float32),
            "b2r": np.asarray(inputs["b2"], np.float32).reshape(1, D).astype(mnp),
            "bf1": np.asarray(inputs["bf1"], np.float32),
            "bf2r": np.asarray(inputs["bf2"], np.float32).reshape(1, D_OUT).astype(mnp),
            "idx1": p1["per_core"][c]["idx"],
            "wsl1": p1["per_core"][c]["wsl"],
            "idx2": p2["per_core"][c]["idx"],
            "wsl2": p2["per_core"][c]["wsl"],
        })

    res = run_bass_kernel_spmd(nc, in_maps, core_ids=list(range(NCORES)),
                               trace=trace)
    out = np.concatenate([res.results[c]["out"][:RPC] for c in range(NCORES)], axis=0)
    return out, res


def kernel(**inputs):
    out, _ = _run(inputs, trace=False)
    return out


# revision 44
# speedup vs baseline: 2.1862x; 1.2691x over previous
"""GCN (2x GCNConv + MLP head + softmax) on 8 TRN2 NeuronCores.

Strategy (graph/data parallel, per sharding hint):
  - Nodes sharded across 8 cores (2500 rows each, padded to 2560); weights
    replicated. Edges partitioned by dst; aggregation runs per dst window
    (128 slots) as dma_gather row gathers (4 SWDGE queues; the gather
    stream is the roofline resource at ~43 GB/s/queue) + one-hot segment
    matmuls on the TensorEngine accumulating in PSUM; fp8 chunk pairs run
    as DoubleRow matmuls (virtual K=256). One-hot matrices are built
    on-chip (DVE iota==slot); self-loops never gather (identity matmul of
    resident rows); trailing pad slots carry idx=-1 so the gather ucode
    skips their descriptors (num_idxs_reg = per-call live count, equalized
    across cores so the SPMD program stays uniform).
  - Layer 1 needs NO AllGather: aggregation commutes with the dense matmul
    (Ahat @ (x W1) == (Ahat @ x) W1), so the host stages the full
    dinv-scaled x as an fp8 table and gathers start immediately. Per
    window: aggregate -> dinv-scale on the PSUM->SBUF copy -> transpose ->
    dense @W1 emitting z1T blocks directly (bias+relu on the Scalar
    engine), software-pipelined one window behind the aggregation.
  - Layer 2 AllGathers the dinv-scaled h2 table in two parts (A=2048
    rows/core fired after z1 window 15, B=512 after window 19) so ~80% of
    its gathers never wait on the last z1 windows. Pass 1 aggregates
    region-A sources for all windows (spilling partials to SBUF), pass 2
    re-adds the spill via identity matmul, folds the bias in as a rank-1
    (sqrt(deg) x bias) matmul, and fuses the final dinv scaling into the
    Scalar-engine relu.
  - Head: two dense layers + row softmax per 128-row window of layer 2.

Host-side preprocessing is limited to graph-structure work (edge sort,
degree counts, window slots, gather-index layout, dinv*x staging).
"""

import os
from contextlib import ExitStack

import numpy as np

import concourse.bacc as bacc
import concourse.mybir as mybir
import concourse.tile as tile
from concourse.bass_utils import run_bass_kernel_spmd
from concourse.masks import make_identity

# problem shapes (hardcoded per contract)
N = 20000
E = 320000
D = 512
D_OUT = 128
NCORES = 8
RPC = 2500          # real rows per core
RPAD = 2560         # padded rows per core (20 tiles of 128)
NPAD = RPAD * NCORES
MT = RPAD // 128    # m-tiles / dst windows per core (20)
SPLIT2 = int(os.environ.get("GNN_SPLIT2", "1280"))  # L2 AG part-A rows/core
RA2 = SPLIT2 * NCORES
MA2 = SPLIT2 // 128
G = 6               # max chunks (of 128 edges) per dma_gather call
MSGB = 12           # msg/onehot ring depth (gather calls in flight)
NQ = 4              # SWDGE queues for gather rotation

# config: "f32" (exact), "f32r" (fast fp32 matmul), "bf16" (half-traffic),
# "fp8" (bf16 compute, fp8e4 gathered tables: quarter gather/AG traffic)
MODE = os.environ.get("GNN_MODE", "fp8")

_f32 = mybir.dt.float32
_f32r = mybir.dt.float32r
_bf16 = mybir.dt.bfloat16
_f8 = mybir.dt.float8e4
_i16 = mybir.dt.int16
_i32 = mybir.dt.int32


def _table_l1(node):
    """Row of node in the host-staged xs table (padded core-major layout)."""
    return (node // RPC) * RPAD + (node % RPC)


def _table_l2(node):
    """Row of node in the L2 AllGather-ed table (part A then part B)."""
    c, r = node // RPC, node % RPC
    h = r >= SPLIT2
    return np.where(h, RA2 + c * (RPAD - SPLIT2) + (r - SPLIT2), c * SPLIT2 + r)


def _plan(tab_ids, dsts, core_bounds, nh, HB):
    """Window/chunk/call plan for one layer's gathers.

    tab_ids: per-edge table row (edges sorted by dst); nh: number of table
    regions (1 for L1, 2 for L2); HB: region-A size (region boundary).
    Returns dict with group_sizes[w][h], chunk_base[w][h], TC,
    vmap[(w,h,ci)], and per-core idx/wsl arrays.
    """
    counts = np.zeros((NCORES, MT, nh), dtype=np.int64)
    for c in range(NCORES):
        lo, hi = core_bounds[c], core_bounds[c + 1]
        d = dsts[lo:hi] - c * RPC
        hvec = (tab_ids[lo:hi] >= HB).astype(np.int64) if nh == 2 else None
        wb = np.searchsorted(d, np.arange(MT + 1) * 128)
        for w in range(MT):
            a, b = wb[w], wb[w + 1]
            if nh == 2:
                n1 = int(hvec[a:b].sum())
                counts[c, w, 0] = (b - a) - n1
                counts[c, w, 1] = n1
            else:
                counts[c, w, 0] = b - a

    cpw = np.maximum(1, -(-counts.max(axis=0) // 128))  # [MT, nh] chunks
    TC = int(cpw.sum())
    cb = np.concatenate([[0], np.cumsum(cpw.reshape(-1))]).astype(int)
    chunk_base = cb[:-1].reshape(MT, nh)

    group_sizes = []
    vmap = {}
    for w in range(MT):
        gw = []
        for h in range(nh):
            n = int(cpw[w, h])
            k = -(-n // G)
            base, rem = divmod(n, k)
            gs = [base + (i < rem) for i in range(k)]
            gw.append(gs)
            off = 0
            for ci, gsz in enumerate(gs):
                lo = off * 128
                v = int(np.clip(counts[:, w, h] - lo, 0, gsz * 128).max())
                vmap[(w, h, ci)] = max(v, 1)
                off += gsz
        group_sizes.append(gw)

    per_core = []
    for c in range(NCORES):
        gidx = np.full((TC, 128), -1, dtype=np.int16)
        wsl = np.full((TC, 128), -1.0, dtype=np.float32)   # dst slot in window
        lo, hi = core_bounds[c], core_bounds[c + 1]
        d = dsts[lo:hi] - c * RPC
        s_ids = tab_ids[lo:hi]
        wb = np.searchsorted(d, np.arange(MT + 1) * 128)
        for w in range(MT):
            a, b = wb[w], wb[w + 1]
            hv = s_ids[a:b] >= HB if nh == 2 else np.zeros(b - a, dtype=bool)
            for h in range(nh):
                sel = hv if h else ~hv
                sid = (s_ids[a:b][sel] - h * HB).astype(np.int16)
                slot = (d[a:b][sel] - w * 128).astype(np.float32)
                k = np.arange(sid.size)
                tg = chunk_base[w, h] + (k // 128)
                row = k % 128
                gidx[tg, row] = sid
                wsl[tg, row] = slot
                # equalize live counts across cores: pad with idx 0 (masked
                # by wsl=-1) up to the call's uniform count; the rest stay -1
                off = 0
                for ci, gsz in enumerate(group_sizes[w][h]):
                    p0 = off * 128
                    v = vmap[(w, h, ci)]
                    cnt = int(np.clip(sid.size - p0, 0, gsz * 128))
                    if cnt < v:
                        kk = np.arange(p0 + cnt, p0 + v)
                        gidx[chunk_base[w, h] + (kk // 128), kk % 128] = 0
                    off += gsz
        # wrapped int16 index layout, one block per gather call
        cols = []
        for w in range(MT):
            for h in range(nh):
                t0 = int(chunk_base[w, h])
                for gsz in group_sizes[w][h]:
                    L = gidx[t0:t0 + gsz].reshape(-1)
                    cols.append(np.tile(L.reshape(-1, 16).T, (8, 1)))
                    t0 += gsz
        idx_np = np.ascontiguousarray(np.concatenate(cols, axis=1))
        per_core.append({
            "idx": idx_np,
            "wsl": np.ascontiguousarray(wsl.T),   # [128, TC]
        })
    return {
        "group_sizes": group_sizes, "chunk_base": chunk_base, "TC": TC,
        "vmap": vmap, "per_core": per_core,
    }


def _prepare(edge_index):
    src = np.asarray(edge_index[0], dtype=np.int64)
    dst = np.asarray(edge_index[1], dtype=np.int64)
    order = np.argsort(dst, kind="stable")
    src, dsts = src[order], dst[order]
    core_bounds = np.searchsorted(dsts, np.arange(NCORES + 1) * RPC)
    p1 = _plan(_table_l1(src).astype(np.int64), dsts, core_bounds, 1, NPAD)
    p2 = _plan(_table_l2(src).astype(np.int64), dsts, core_bounds, 2, RA2)
    return p1, p2


def _build(p1, p2):
    # mdt: matmul-operand dtype; tdt: gathered-table dtype; trdt: transpose dtype
    mdt = {"f32": _f32, "f32r": _f32r, "bf16": _bf16, "fp8": _bf16}[MODE]
    tdt = {"f32": _f32, "f32r": _f32, "bf16": _bf16, "fp8": _f8}[MODE]
    trdt = _bf16 if MODE in ("bf16", "fp8") else _f32
    # DoubleRow: two fp8 128-edge chunks per matmul (virtual K=256)
    DR = MODE == "fp8" and os.environ.get("GNN_DR", "1") == "1"
    TC1, TC2 = p1["TC"], p2["TC"]

    nc = bacc.Bacc("TRN2", target_bir_lowering=False, debug=False,
                   num_devices=NCORES, num_swdge_queues=NQ)
    xs_d = nc.dram_tensor("xs", [NPAD, D], tdt, kind="ExternalInput")
    xsk_d = nc.dram_tensor("xsk", [RPAD, D], tdt, kind="ExternalInput")
    dinv_d = nc.dram_tensor("dinv", [RPAD], _f32, kind="ExternalInput")
    W_d = {k: nc.dram_tensor(k, [D, D], mdt, kind="ExternalInput")
           for k in ("W1", "W2", "Wf1")}
    Wf2_d = nc.dram_tensor("Wf2", [D, D_OUT], mdt, kind="ExternalInput")
    b1_d = nc.dram_tensor("b1", [D], _f32, kind="ExternalInput")
    b2r_d = nc.dram_tensor("b2r", [1, D], mdt, kind="ExternalInput")
    sqd_d = nc.dram_tensor("sqd", [1, RPAD], mdt, kind="ExternalInput")
    bf1_d = nc.dram_tensor("bf1", [D], _f32, kind="ExternalInput")
    bf2r_d = nc.dram_tensor("bf2r", [1, D_OUT], mdt, kind="ExternalInput")
    idx1_d = nc.dram_tensor("idx1", [128, TC1 * 8], _i16, kind="ExternalInput")
    wsl1_d = nc.dram_tensor("wsl1", [128, TC1], _f32, kind="ExternalInput")
    idx2_d = nc.dram_tensor("idx2", [128, TC2 * 8], _i16, kind="ExternalInput")
    wsl2_d = nc.dram_tensor("wsl2", [128, TC2], _f32, kind="ExternalInput")
    out_d = nc.dram_tensor("out", [RPAD, D_OUT], _f32, kind="ExternalOutput")

    # L2 AllGather: part A/B as separate tensors so each AllGather's input
    # dependency covers only the phase-A writes that actually feed it
    cc_inA = nc.dram_tensor("cc_inA", [SPLIT2, D], tdt, kind="Internal")
    cc_inB = nc.dram_tensor("cc_inB", [RPAD - SPLIT2, D], tdt, kind="Internal")
    cc_out = nc.dram_tensor("cc_out", [NPAD, D], tdt, kind="Internal",
                            addr_space="Shared")

    RG = [list(range(NCORES))]
    ACT = mybir.ActivationFunctionType
    ALU = mybir.AluOpType

    with tile.TileContext(nc) as tc, ExitStack() as ctx:
        const = ctx.enter_context(tc.tile_pool(name="const", bufs=1))
        actT = ctx.enter_context(tc.tile_pool(name="actT", bufs=2))
        work = ctx.enter_context(tc.tile_pool(name="work", bufs=2))
        aggp = ctx.enter_context(tc.tile_pool(name="aggp", bufs=2))
        msgp = ctx.enter_context(tc.tile_pool(name="msgp", bufs=MSGB))
        sp = ctx.enter_context(tc.tile_pool(name="sp", bufs=MSGB))
        spillp = ctx.enter_context(tc.tile_pool(name="spillp", bufs=MT))
        psA = ctx.enter_context(tc.tile_pool(name="psA", bufs=2, space="PSUM"))
        psC = ctx.enter_context(tc.tile_pool(name="psC", bufs=4, space="PSUM"))
        psT = ctx.enter_context(tc.tile_pool(name="psT", bufs=2, space="PSUM"))

        # ---- constants; L1 gather prerequisites (idx1/wsl1) first ----
        idx1_t = const.tile([128, TC1 * 8], _i16)
        nc.sync.dma_start(idx1_t[:], idx1_d.ap())
        wsl1_t = const.tile([128, TC1], _f32)
        nc.sync.dma_start(wsl1_t[:], wsl1_d.ap())
        dinv_t = const.tile([128, MT], _f32)
        nc.sync.dma_start(dinv_t[:], dinv_d.ap().rearrange("(a p) -> p a", p=128))
        xsk_t = const.tile([128, MT, D], tdt)
        nc.sync.dma_start(xsk_t[:], xsk_d.ap().rearrange("(m p) f -> p m f", p=128))
        w_t = {}
        for k in ("W1", "W2", "Wf1"):
            w_t[k] = const.tile([128, 4, D], mdt, name=f"wt_{k}")
        nc.sync.dma_start(w_t["W1"][:], W_d["W1"].ap().rearrange("(k p) n -> p k n", p=128))
        b1c_t = const.tile([128, 4], _f32)
        nc.sync.dma_start(b1c_t[:], b1_d.ap().rearrange("(a p) -> p a", p=128))

        # the rest on the Activation-engine HWDGE queue (Sync stays free)
        nc.scalar.dma_start(w_t["W2"][:], W_d["W2"].ap().rearrange("(k p) n -> p k n", p=128))
        nc.scalar.dma_start(w_t["Wf1"][:], W_d["Wf1"].ap().rearrange("(k p) n -> p k n", p=128))
        wf2_t = const.tile([128, 4, D_OUT], mdt)
        nc.scalar.dma_start(wf2_t[:], Wf2_d.ap().rearrange("(k p) n -> p k n", p=128))
        b2r_t = const.tile([1, D], mdt)
        nc.scalar.dma_start(b2r_t[:], b2r_d.ap())
        sqd_t = const.tile([1, RPAD], mdt)
        nc.scalar.dma_start(sqd_t[:], sqd_d.ap())
        bf1_t = const.tile([128, 4], _f32)
        nc.scalar.dma_start(bf1_t[:], bf1_d.ap().rearrange("(a p) -> p a", p=128))
        bf2r_t = const.tile([1, D_OUT], mdt)
        nc.scalar.dma_start(bf2r_t[:], bf2r_d.ap())
        idx2_t = const.tile([128, TC2 * 8], _i16)
        nc.scalar.dma_start(idx2_t[:], idx2_d.ap())
        wsl2_t = const.tile([128, TC2], _f32)
        nc.scalar.dma_start(wsl2_t[:], wsl2_d.ap())

        ones_t = const.tile([1, 128], mdt)
        nc.vector.memset(ones_t[:], 1.0)
        hsk2_t = const.tile([128, MT, D], tdt, name="hsk2")
        ident = const.tile([128, 128], trdt)
        make_identity(nc, ident[:])
        iota_i = const.tile([128, G, 128], _i32)
        nc.gpsimd.iota(iota_i[:], pattern=[[0, G], [1, 128]], base=0, channel_multiplier=0)
        iota_h = const.tile([128, G, 128], _f32)
        nc.vector.tensor_copy(iota_h[:], iota_i[:])

        # zero all msg ring buffers once: gather calls skip trailing pad
        # descriptors, and 0-coefficient masking in the matmul still NaNs on
        # uninitialized SBUF (0 x NaN), so stale rows must start finite
        for _ in range(MSGB):
            m0 = msgp.tile([128, G, D], tdt, tag="msg")
            nc.vector.memset(m0[:], 0.0)

        qload = [0] * NQ

        def gather_half(plan, idx_t, wsl_t, src_ap, w, h, ps, tail_mm):
            """Gathers + one-hot chunk matmuls for window w, region h."""
            group_sizes, chunk_base, vmap = (
                plan["group_sizes"], plan["chunk_base"], plan["vmap"])
            t0 = int(chunk_base[w][h])
            col0 = t0 * 8
            if DR:
                n_chunk = sum((g // 2) + (g % 2) for g in group_sizes[w][h])
            else:
                n_chunk = sum(group_sizes[w][h])
            n = n_chunk + len(tail_mm)
            done = 0
            for ci, gsz in enumerate(group_sizes[w][h]):
                msg = msgp.tile([128, G, D], tdt, tag="msg")
                q = min(range(NQ), key=lambda i: qload[i])
                qload[q] += vmap[(w, h, ci)]
                nc.gpsimd.dma_gather(msg[:, :gsz, :], src_ap,
                                     idx_t[:, col0:col0 + gsz * 8],
                                     gsz * 128, vmap[(w, h, ci)], D,
                                     queue_num=q)
                s_t = sp.tile([128, G, 128], tdt, tag="S")
                nc.vector.tensor_tensor(
                    s_t[:, :gsz, :], iota_h[:, :gsz, :],
                    wsl_t[:, t0:t0 + gsz].to_broadcast([128, gsz, 128]),
                    op=ALU.is_equal)
                t = 0
                while t < gsz:
                    if DR and t + 1 < gsz:
                        nc.tensor.matmul(
                            ps[:], lhsT=s_t[:, t:t + 2, :],
                            rhs=msg[:, t:t + 2, :],
                            start=(done == 0), stop=(done == n - 1),
                            perf_mode=mybir.MatmulPerfMode.DoubleRow)
                        t += 2
                    else:
                        nc.tensor.matmul(ps[:], lhsT=s_t[:, t, :],
                                         rhs=msg[:, t, :],
                                         start=(done == 0), stop=(done == n - 1))
                        t += 1
                    done += 1
                t0 += gsz
                col0 += gsz * 8
            for lhsT, rhs in tail_mm:
                nc.tensor.matmul(ps[:], lhsT=lhsT, rhs=rhs,
                                 start=False, stop=(done == n - 1))
                done += 1

        # ================= layer 1: gather-then-dense, no AllGather ========
        z1T = actT.tile([128, 4, RPAD], mdt, tag="zT")
        src1 = xs_d.ap().bitcast(tdt)

        def l1_dense(w, aggdT):
            # z1T blocks for window w: relu(W1.T-block @ aggdT + b1)
            for q in range(4):
                pd = psA.tile([128, 128], _f32, tag="psA", name="pd")
                for k in range(4):
                    nc.tensor.matmul(pd[:],
                                     lhsT=w_t["W1"][:, k, q * 128:(q + 1) * 128],
                                     rhs=aggdT[:, k, :],
                                     start=(k == 0), stop=(k == 3))
                nc.scalar.activation(z1T[:, q, w * 128:(w + 1) * 128], pd[:],
                                     ACT.Relu, bias=b1c_t[:, q:q + 1])
            # phase A of layer 2 for this m-tile: hs2 = dinv * (z1 @ W2)
            ps = psA.tile([128, D], _f32, tag="psA")
            for k in range(4):
                nc.tensor.matmul(ps[:], lhsT=z1T[:, k, w * 128:(w + 1) * 128],
                                 rhs=w_t["W2"][:, k, :], start=(k == 0), stop=(k == 3))
            nc.scalar.activation(hsk2_t[:, w, :], ps[:], ACT.Copy,
                                 scale=dinv_t[:, w:w + 1])
            part, r0 = (cc_inA, 0) if w < MA2 else (cc_inB, SPLIT2)
            nc.scalar.dma_start(
                part.ap()[w * 128 - r0:(w + 1) * 128 - r0, :], hsk2_t[:, w, :])
            if w == MA2 - 1:
                nc.gpsimd.collective_compute(
                    "AllGather", ALU.bypass, ins=[cc_inA.ap()],
                    outs=[cc_out.ap()[0:RA2]], replica_groups=RG)
            elif w == MT - 1:
                nc.gpsimd.collective_compute(
                    "AllGather", ALU.bypass, ins=[cc_inB.ap()],
                    outs=[cc_out.ap()[RA2:NPAD]], replica_groups=RG)

        pending = None  # (w, aggdT): dense deferred one window for overlap
        for w in range(MT):
            ps = psC.tile([128, D], _f32, tag="psC", name=f"agg_{w}")
            gather_half(p1, idx1_t, wsl1_t, src1, w, 0, ps,
                        [(ident[:], xsk_t[:, w, :])])
            # dinv-scale on the PSUM->SBUF copy (commutes with @W1)
            aggsb = work.tile([128, D], trdt, tag="aggsb")
            nc.scalar.activation(aggsb[:], ps[:], ACT.Copy,
                                 scale=dinv_t[:, w:w + 1])
            aggdT = aggp.tile([128, 4, 128], trdt, tag="aggdT")
            for q in range(4):
                pt = psT.tile([128, 128], trdt, tag="psT")
                nc.tensor.transpose(pt[:], aggsb[:, q * 128:(q + 1) * 128], ident[:])
                nc.vector.tensor_copy(aggdT[:, q, :], pt[:])
            if pending is not None:
                l1_dense(*pending)
            pending = (w, aggdT)
        l1_dense(*pending)

        # ================= layer 2 aggregation + head ======================
        z2T = actT.tile([128, 4, RPAD], mdt, tag="zT")

        def head_window(m):
            z3w = work.tile([128, 4, 128], mdt, tag="z3w")
            for q in range(4):
                ps = psA.tile([128, D], _f32, tag="psA")
                for k in range(4):
                    nc.tensor.matmul(ps[:, 0:128],
                                     lhsT=w_t["Wf1"][:, k, q * 128:(q + 1) * 128],
                                     rhs=z2T[:, k, m * 128:(m + 1) * 128],
                                     start=(k == 0), stop=(k == 3))
                nc.scalar.activation(z3w[:, q, :], ps[:, 0:128],
                                     ACT.Relu, bias=bf1_t[:, q:q + 1])
            ps2 = psT.tile([128, D_OUT], _f32, tag="psT")
            for k in range(4):
                nc.tensor.matmul(ps2[:], lhsT=z3w[:, k, :],
                                 rhs=wf2_t[:, k, :], start=(k == 0), stop=False)
            nc.tensor.matmul(ps2[:], lhsT=ones_t[0:1, :], rhs=bf2r_t[0:1, :],
                             start=False, stop=True)
            nmx = work.tile([128, 1], _f32, tag="nmx")
            nc.vector.tensor_reduce(nmx[:], ps2[:], axis=mybir.AxisListType.X,
                                    op=ALU.max, negate=True)
            ex = work.tile([128, D_OUT], _f32, tag="ex")
            sm = work.tile([128, 1], _f32, tag="sm")
            nc.scalar.activation(ex[:], ps2[:], ACT.Exp, bias=nmx[:, :1], scale=1.0,
                                 accum_out=sm[:, :1])
            rin = work.tile([128, 1], _f32, tag="rin")
            nc.vector.reciprocal(rin[:], sm[:])
            ot = work.tile([128, D_OUT], _f32, tag="ot")
            nc.vector.tensor_tensor(ot[:], ex[:], rin[:, :1].to_broadcast([128, D_OUT]),
                                    op=ALU.mult)
            nc.sync.dma_start(out_d.ap()[m * 128:(m + 1) * 128, :], ot[:])

        spills = {}
        srcA = cc_out.ap()[0:RA2].bitcast(tdt)
        srcB = cc_out.ap()[RA2:NPAD].bitcast(tdt)
        for w in range(MT):
            ps = psC.tile([128, D], _f32, tag="psC", name=f"ps1_{w}")
            gather_half(p2, idx2_t, wsl2_t, srcA, w, 0, ps,
                        [(ident[:], hsk2_t[:, w, :])])
            sp_w = spillp.tile([128, D], trdt, tag="spill", name=f"spill_{w}")
            nc.scalar.copy(sp_w[:], ps[:])
            spills[w] = sp_w

        for w in range(MT):
            ps = psC.tile([128, D], _f32, tag="psC", name=f"ps2_{w}")
            gather_half(p2, idx2_t, wsl2_t, srcB, w, 1, ps, [
                (ident[:], spills[w][:]),
                (sqd_t[0:1, w * 128:(w + 1) * 128], b2r_t[0:1, :]),
            ])
            zrel = work.tile([128, D], trdt, tag="zrel")
            nc.scalar.activation(zrel[:], ps[:], ACT.Relu,
                                 scale=dinv_t[:, w:w + 1])
            for q in range(4):
                pt = psT.tile([128, 128], trdt, tag="psT")
                nc.tensor.transpose(pt[:], zrel[:, q * 128:(q + 1) * 128], ident[:])
                nc.vector.tensor_copy(z2T[:, q, w * 128:(w + 1) * 128], pt[:])
            head_window(w)

    nc.compile()
    return nc


def _run(inputs, trace=False):
    x = np.asarray(inputs["x"], dtype=np.float32)
    edge_index = np.asarray(inputs["edge_index"])
    deg = np.bincount(
        np.concatenate([edge_index[1], np.arange(N, dtype=edge_index.dtype)]),
        minlength=N,
    ).astype(np.float32)
    dinv = np.zeros(N, dtype=np.float32)
    nz = deg > 0
    dinv[nz] = (1.0 / np.sqrt(deg[nz])).astype(np.float32)
    sqd = np.zeros(N, dtype=np.float32)
    sqd[nz] = np.sqrt(deg[nz]).astype(np.float32)

    p1, p2 = _prepare(edge_index)
    nc = _build(p1, p2)

    import ml_dtypes
    if MODE in ("bf16", "fp8"):
        mnp = ml_dtypes.bfloat16
    else:
        mnp = np.float32
    tnp = {"f32": np.float32, "f32r": np.float32, "bf16": ml_dtypes.bfloat16,
           "fp8": ml_dtypes.float8_e4m3}[MODE]

    # full dinv-scaled x table in the padded core-major layout
    xs_full = np.zeros((NPAD, D), dtype=np.float32)
    for c in range(NCORES):
        xs_full[c * RPAD:c * RPAD + RPC] = (
            dinv[c * RPC:(c + 1) * RPC, None] * x[c * RPC:(c + 1) * RPC])
    xs_full = np.ascontiguousarray(xs_full).astype(tnp)

    in_maps = []
    for c in range(NCORES):
        dv = np.zeros(RPAD, dtype=np.float32)
        dv[:RPC] = dinv[c * RPC:(c + 1) * RPC]
        sq = np.zeros(RPAD, dtype=np.float32)
        sq[:RPC] = sqd[c * RPC:(c + 1) * RPC]
        in_maps.append({
            "xs": xs_full,
            "xsk": np.ascontiguousarray(xs_full[c * RPAD:(c + 1) * RPAD]),
            "dinv": dv,
            "sqd": sq.reshape(1, RPAD).astype(mnp),
            "W1": np.asarray(inputs["W1"], np.float32).astype(mnp),
            "W2": np.asarray(inputs["W2"], np.float32).astype(mnp),
            "Wf1": np.asarray(inputs["Wf1"], np.float32).astype(mnp),
            "Wf2": np.asarray(inputs["Wf2"], np.float32).astype(mnp),
            "b1": np.asarray(inputs["b1"], np.float32),
            "b2r": np.asarray(inputs["b2"], np.float32).reshape(1, D).astype(mnp),
            "bf1": np.asarray(inputs["bf1"], np.float32),
            "bf2r": np.asarray(inputs["bf2"], np.float32).reshape(1, D_OUT).astype(mnp),
            "idx1": p1["per_core"][c]["idx"],
            "wsl1": p1["per_core"][c]["wsl"],
            "idx2": p2["per_core"][c]["idx"],
            "wsl2": p2["per_core"][c]["wsl"],
        })

    res = run_bass_kernel_spmd(nc, in_maps, core_ids=list(range(NCORES)),
                               trace=trace)
    out = np.concatenate([res.results[c]["out"][:RPC] for c in range(NCORES)], axis=0)
    return out, res


def kernel(**inputs):
    out, _ = _run(inputs, trace=False)
    return out
